# revision 1
# baseline (speedup 1.0000x reference)
"""Trainium2 Bass kernel for nn_Block_15144054685914 (dense transformer block).

Sharding: 8 cores = 2 batch groups (DP) x 4-way tensor parallel.
  core c: batch b = c//4, heads [4*(c%4), 4*(c%4)+4), FFN slice c%4.
One on-device bf16 AllReduce per t-chunk (attention residual) within each
4-core batch group; final partial outputs summed on host.

Math tricks (all exact up to float rounding):
  - rmsnorm(x) scale cancels for Q/K (rmsnorm(rope(c*v)) == rmsnorm(rope(v)))
  - rmsnorm scale for the MLP folds into a per-row s2^2 post-scale
  - softmax 1/sum folds into a post-PV column scale; row sums via ones-matmul
  - alpha softmax mixing done on-chip with scaled-identity matmuls
"""

import math
import numpy as np
import ml_dtypes

B, E, H, J = 2, 2048, 16, 4
D = 128
GC = 12
FF = 4 * E
NCORES = 8
HL = H // 4            # local heads per core
HDL = HL * D           # 512
JD = J * D             # 512
FL = FF // 4           # 2048 local ffn rows
EPS = float(np.finfo(np.float32).eps)
T_FULL = 2048
CH = 512               # t-chunk for attention + AllReduce
EC = E // 128          # 16
FCT = FL // 128        # 16 f-tiles

bf16n = ml_dtypes.bfloat16
DBG = False
NOAR = False


def _bf(x):
    return np.ascontiguousarray(np.asarray(x, dtype=np.float32)).astype(bf16n)


def shard_inputs(x, ve, cos, sin, Wq, Wk, Wv, Wo, alpha_k, alpha_v, Wg,
                 Wfc, Wmlp, T=T_FULL):
    x = np.asarray(x, np.float32)[:, :T]
    ve = np.asarray(ve, np.float32)[:, :T]
    cosf = np.asarray(cos, np.float32)[0, :T, 0, :]   # (T, 64)
    sinf = np.asarray(sin, np.float32)[0, :T, 0, :]
    Wq = np.asarray(Wq, np.float32)
    Wk = np.asarray(Wk, np.float32)
    Wv = np.asarray(Wv, np.float32)
    Wo = np.asarray(Wo, np.float32)
    Wg = np.asarray(Wg, np.float32)
    Wfc = np.asarray(Wfc, np.float32)
    Wmlp = np.asarray(Wmlp, np.float32)
    alpha_k = np.asarray(alpha_k, np.float32)
    alpha_v = np.asarray(alpha_v, np.float32)

    nch = T // CH
    # causal 0/1 mask patterns for the 4 s-blocks crossing the diagonal of a
    # 512-wide t-chunk: masks[m][s, t] = 1 if (m*128 + s) <= t
    masks = np.zeros((4, 128, CH), np.float32)
    for m in range(4):
        s_idx = np.arange(128)[:, None] + m * 128
        t_idx = np.arange(CH)[None, :]
        masks[m] = (s_idx <= t_idx).astype(np.float32)

    in_maps = []
    for c in range(NCORES):
        b = c // 4
        hg = c % 4
        hsl = slice(hg * HDL, (hg + 1) * HDL)      # head-dim slice of E/heads
        fsl = slice(hg * FL, (hg + 1) * FL)        # ffn slice
        m = {
            "xq": _bf(0.25 * x[b]),                            # (T, E)
            "xT": _bf(x[b].T),                                 # (E, T)
            "veT": _bf(ve[b].T),                               # (JD, T)
            "cos2": _bf(np.concatenate([cosf.T, cosf.T], 0)),  # (128, T)
            "sin2": _bf(np.concatenate([sinf.T, -sinf.T], 0)),  # (128, T)
            "p64": _bf(np.eye(128)[:, list(range(64, 128)) + list(range(64))].T),
            "wqT": _bf(Wq[hsl, :].T),                          # (E, HDL)
            "wkT": _bf(Wk.T),                                  # (E, JD)
            "wvT": _bf(Wv.T),                                  # (E, JD)
            "woT": _bf(Wo.T[hsl, :]),                          # (HDL, E)
            "wfcT": _bf(Wfc.T[:, fsl]),                        # (E, FL)
            "wmlpT": _bf(Wmlp.T[fsl, :]),                      # (FL, E)
            "wgT": _bf(Wg[hg * HL:(hg + 1) * HL, :].T),        # (GC, HL)
            "ak1": np.ascontiguousarray(
                alpha_k[hg * HL:(hg + 1) * HL, :].reshape(1, HL * J)),
            "av1": np.ascontiguousarray(
                alpha_v[hg * HL:(hg + 1) * HL, :].reshape(1, HL * J)),
            "masks": _bf(masks),                               # (4, 128, CH)
            "ident": _bf(np.eye(128)),
            "onec": _bf(np.ones((128, 1))),
            "oner": _bf(np.ones((1, 128))),
        }
        in_maps.append(m)
    return in_maps


def declare_io(nc, T):
    import concourse.mybir as mybir
    bf = mybir.dt.bfloat16
    f32 = mybir.dt.float32
    io = {}

    def inp(name, shape, dt=bf):
        io[name] = nc.dram_tensor(name, list(shape), dt, kind="ExternalInput").ap()

    inp("xq", (T, E)); inp("xT", (E, T)); inp("veT", (JD, T))
    inp("cos2", (128, T)); inp("sin2", (128, T)); inp("p64", (128, 128))
    inp("wqT", (E, HDL)); inp("wkT", (E, JD)); inp("wvT", (E, JD))
    inp("woT", (HDL, E)); inp("wfcT", (E, FL)); inp("wmlpT", (FL, E))
    inp("wgT", (GC, HL))
    inp("ak1", (1, HL * J), f32); inp("av1", (1, HL * J), f32)
    inp("masks", (4, 128, CH)); inp("ident", (128, 128))
    inp("onec", (128, 1)); inp("oner", (1, 128))
    io["out"] = nc.dram_tensor("out", [T, E], bf, kind="ExternalOutput").ap()
    io["out_x1"] = nc.dram_tensor("out_x1", [T, E], bf, kind="ExternalOutput").ap()
    return io


def emit(tc, io, T):
    import concourse.bass as bass
    import concourse.mybir as mybir
    from contextlib import ExitStack

    nc = tc.nc
    bf = mybir.dt.bfloat16
    f32 = mybir.dt.float32
    AF = mybir.ActivationFunctionType
    OP = mybir.AluOpType
    nch = T // CH
    TT = T // 128                  # number of 128-row t-tiles
    qk_ln_scale = 1.0 / (128.0 * 1.44)   # mean over D and the 1.2^2 fold
    inv_sqrt_d = 1.0 / math.sqrt(D)

    with ExitStack() as ctx:
        cpool = ctx.enter_context(tc.tile_pool(name="const", bufs=1))
        big = ctx.enter_context(tc.tile_pool(name="big", bufs=1))
        wk = ctx.enter_context(tc.tile_pool(name="wk", bufs=1))
        colp = ctx.enter_context(tc.tile_pool(name="colp", bufs=1))
        psp = ctx.enter_context(tc.tile_pool(name="psp", bufs=1, space="PSUM"))
        dram = ctx.enter_context(tc.tile_pool(name="dram", bufs=2, space="DRAM"))

        # ---------------- chunk-0 stream prefetch ----------------
        xt_tiles = {}

        def load_xt(ci):
            t = big.tile([128, EC, CH], bf, name=f"xt{ci}", tag="xt", bufs=2)
            nc.sync.dma_start(
                t[:], io["xT"].rearrange("(a p) t -> p a t", p=128)
                [:, :, ci * CH:(ci + 1) * CH])
            xt_tiles[ci] = t

        load_xt(0)
        wpre = {}
        for tag, wio_name in (("kj", "wkT"), ("vj", "wvT")):
            for jd in range(4):
                wt0 = wk.tile([128, EC, 128], bf, name=f"w{tag}0_{jd}",
                              tag="we3", bufs=5)
                nc.sync.dma_start(
                    wt0[:], io[wio_name].rearrange("(a p) n -> p a n", p=128)
                    [:, :, jd * 128:(jd + 1) * 128])
                wpre[(tag, jd)] = wt0

        # ---------------- constants ----------------
        ident = cpool.tile([128, 128], bf)
        nc.sync.dma_start(ident[:], io["ident"][:])
        onec = cpool.tile([128, 1], bf)
        nc.sync.dma_start(onec[:], io["onec"][:])
        oner = cpool.tile([1, 128], bf)
        nc.sync.dma_start(oner[:], io["oner"][:])
        masks = cpool.tile([128, 4, CH], bf)
        nc.sync.dma_start(masks[:], io["masks"].rearrange("m p n -> p m n"))

        p64 = cpool.tile([128, 128], bf)
        nc.sync.dma_start(p64[:], io["p64"][:])
        wgT = cpool.tile([GC, HL], bf)
        nc.sync.dma_start(wgT[:], io["wgT"][:])
        eps_e = cpool.tile([128, 1], f32)
        nc.vector.memset(eps_e[:], EPS)
        eps_qk = cpool.tile([1, 1], f32)
        nc.vector.memset(eps_qk[:], EPS / 1.44)

        # ---------------- alpha softmax + mix matrices ----------------
        def softmax16(name, src):
            a1 = cpool.tile([1, HL * J], f32, name=f"{name}_a1")
            nc.sync.dma_start(a1[:], src[:])
            e1 = cpool.tile([1, HL * J], f32, name=f"{name}_e1")
            nc.scalar.activation(e1[:], a1[:], AF.Exp)
            w1 = cpool.tile([1, HL * J], bf, name=f"{name}_w1")
            sme = cpool.tile([1, HL], f32, name=f"{name}_sme")
            nc.vector.tensor_reduce(sme[:],
                                    e1[:].rearrange("p (a b) -> p a b", b=J),
                                    axis=mybir.AxisListType.X, op=OP.add)
            rse = cpool.tile([1, HL], f32, name=f"{name}_rse")
            nc.vector.reciprocal(rse[:], sme[:])
            for h in range(HL):
                sl = slice(h * J, (h + 1) * J)
                nc.vector.tensor_scalar(w1[0:1, sl], e1[0:1, sl],
                                        rse[0:1, h:h + 1], None, op0=OP.mult)
            # broadcast to 128 partitions: (128 x 16) = oner.T @ w1
            wb_ps = psp.tile([128, HL * J], f32, name=f"{name}_wbps", tag="ps",
                             bufs=8)
            nc.tensor.matmul(wb_ps[:], oner[:], w1[:], start=True, stop=True)
            wb = cpool.tile([128, HL * J], f32, name=f"{name}_wb")
            nc.scalar.copy(wb[:], wb_ps[:])
            return wb

        wkb_b = softmax16("ak", io["ak1"])   # (128, 16) cols h*J+j
        wvb_b = softmax16("av", io["av1"])

        # k-mix scaled identities kI[h*J+j] and v/ve block matrices Bv[j]
        kI = cpool.tile([128, HL * J, 128], bf)
        Bv = cpool.tile([128, J, HDL], bf)
        Bev = cpool.tile([128, J, HDL], bf)
        for h in range(HL):
            for j in range(J):
                nc.vector.tensor_scalar(kI[:, h * J + j, :], ident[:],
                                        wkb_b[:, h * J + j:h * J + j + 1], None,
                                        op0=OP.mult)
                nc.vector.tensor_scalar(Bv[:, j, h * D:(h + 1) * D], ident[:],
                                        wvb_b[:, h * J + j:h * J + j + 1], None,
                                        op0=OP.mult)
                nc.vector.tensor_scalar(Bev[:, j, h * D:(h + 1) * D], ident[:],
                                        wvb_b[:, h * J + j:h * J + j + 1], None,
                                        op0=OP.mult)

        kT = big.tile([128, HL, T], bf)           # final K, feature-major
        vtile = big.tile([128, TT, HDL], bf)      # final V, token-major

        cin = dram.tile([T, E], bf)
        cout = dram.tile([T, E], bf)

        scols = []     # per t-tile rmsnorm(x) scale (128,1) f32
        s2cols = []    # per t-tile s2^2 (128,1) f32

        groups = [[0, 1, 2, 3], [4, 5, 6, 7]]

        def row_stats_sq(x_tt, name):
            """mean of squares per row of a (128, E) bf16 tile -> (128,1) f32."""
            bnt = colp.tile([128, 4, 6], f32, name=f"{name}_bnt", tag="bnt",
                            bufs=2)
            for i in range(4):
                nc.vector.bn_stats(bnt[:, i, :],
                                   x_tt[:, i * 512:(i + 1) * 512])
            agg = colp.tile([128, 2], f32, name=f"{name}_agg", tag="agg",
                            bufs=2)
            nc.vector.bn_aggr(agg[:], bnt[:])
            m2 = colp.tile([128, 1], f32, name=f"{name}_m2", tag="c1", bufs=8)
            nc.vector.tensor_tensor(m2[:], agg[:, 0:1], agg[:, 0:1], op=OP.mult)
            msq = colp.tile([128, 1], f32, name=f"{name}_msq", tag="c1", bufs=8)
            nc.vector.tensor_tensor(msq[:], m2[:], agg[:, 1:2], op=OP.add)
            return msq

        def w_etile(src_io, cols, name):
            t = wk.tile([128, cols], bf, name=name, tag="we", bufs=10)
            return t

        # ======================= attention phase =======================
        for c in range(nch):
            csl = slice(c * CH, (c + 1) * CH)
            cos2 = wk.tile([128, CH], bf, name=f"cos2_{c}", tag="cs", bufs=2)
            nc.sync.dma_start(cos2[:], io["cos2"][:, csl])
            sin2 = wk.tile([128, CH], bf, name=f"sin2_{c}", tag="cs", bufs=2)
            nc.sync.dma_start(sin2[:], io["sin2"][:, csl])
            xt = xt_tiles.pop(c)

            # xq t-tiles + s[t] = rsqrt(mean(x^2)+eps) = exp(-0.5*ln(.))
            for tt in range(4):
                rows = slice(c * CH + tt * 128, c * CH + (tt + 1) * 128)
                xq_tt = wk.tile([128, E], bf, name=f"xq{c}_{tt}", tag="xq",
                                bufs=2)
                nc.sync.dma_start(xq_tt[:], io["xq"][rows, :])
                msq = row_stats_sq(xq_tt, f"s{c}_{tt}")
                lnm = colp.tile([128, 1], f32, name=f"lnm{c}_{tt}", tag="c1",
                                bufs=8)
                # mean(x^2) = 16*msq  (xq = x/4)
                nc.scalar.activation(lnm[:], msq[:], AF.Ln, scale=16.0,
                                     bias=eps_e[:])
                scol = colp.tile([128, 1], f32, name=f"scol{c}_{tt}",
                                 tag="scol", bufs=4 * nch)
                nc.scalar.activation(scol[:], lnm[:], AF.Exp, scale=-0.5)
                scols.append(scol)

            # ---- kj / vj projections (feature-major) ----
            def proj_jd(wio, tag):
                outs = []
                for jd in range(4):
                    if (tag, jd) in wpre and c == 0:
                        wt = wpre[(tag, jd)]
                    else:
                        wt = wk.tile([128, EC, 128], bf,
                                     name=f"w{tag}{c}_{jd}", tag="we3",
                                     bufs=5)
                        nc.sync.dma_start(
                            wt[:], wio.rearrange("(a p) n -> p a n", p=128)
                            [:, :, jd * 128:(jd + 1) * 128])
                    ps = psp.tile([128, CH], f32, name=f"p{tag}{c}_{jd}",
                                  tag="ps", bufs=8)
                    for e in range(EC):
                        nc.tensor.matmul(ps[:], wt[:, e, :], xt[:, e, :],
                                         start=(e == 0), stop=(e == EC - 1))
                    sb = wk.tile([128, CH], bf, name=f"s{tag}{c}_{jd}",
                                 tag="sb", bufs=7)
                    nc.scalar.copy(sb[:], ps[:])
                    outs.append(sb)
                return outs

            kj_sb = proj_jd(io["wkT"], "kj")
            vj_sb = proj_jd(io["wvT"], "vj")

            # ---- gate (token-major) ----
            g3s = []
            for tt in range(4):
                tsl = slice(tt * 128, (tt + 1) * 128)
                g_ps = psp.tile([128, HL], f32, name=f"gps{c}_{tt}", tag="ps",
                                bufs=8)
                nc.tensor.matmul(g_ps[:], xt[0:GC, 0, tsl], wgT[:],
                                 start=True, stop=True)
                zs = colp.tile([128, HL], f32, name=f"zs{c}_{tt}", tag="g4",
                               bufs=3)
                nc.vector.tensor_scalar(zs[:], g_ps[:], scols[c * 4 + tt][:],
                                        None, op0=OP.mult)
                ge = colp.tile([128, HL], f32, name=f"ge{c}_{tt}", tag="g4",
                               bufs=3)
                nc.scalar.activation(ge[:], zs[:], AF.Exp, scale=-1.0)
                gd = colp.tile([128, HL], f32, name=f"gd{c}_{tt}", tag="g4",
                               bufs=3)
                nc.vector.tensor_scalar(gd[:], ge[:], 1.0, None, op0=OP.add)
                gr = colp.tile([128, HL], f32, name=f"gr{c}_{tt}", tag="g4",
                               bufs=3)
                nc.vector.reciprocal(gr[:], gd[:])
                g3 = colp.tile([128, HL], f32, name=f"g3{c}_{tt}", tag="g3",
                               bufs=4)
                nc.vector.tensor_scalar(g3[:], gr[:], 3.0, None, op0=OP.mult)
                g3s.append(g3)

            # ---- V assembly (token-major) ----
            vet = wk.tile([128, J, CH], bf, name=f"vet{c}", tag="we3", bufs=5)
            nc.sync.dma_start(
                vet[:], io["veT"].rearrange("(a p) t -> p a t", p=128)[:, :, csl])
            for tt in range(4):
                tsl = slice(tt * 128, (tt + 1) * 128)
                vm_ps = psp.tile([128, HDL], f32, name=f"vmps{c}_{tt}",
                                 tag="ps", bufs=8)
                ve_ps = psp.tile([128, HDL], f32, name=f"veps{c}_{tt}",
                                 tag="ps", bufs=8)
                for j in range(J):
                    nc.tensor.matmul(vm_ps[:], vj_sb[j][:, tsl], Bv[:, j, :],
                                     start=(j == 0), stop=(j == J - 1))
                    nc.tensor.matmul(ve_ps[:], vet[:, j, tsl], Bev[:, j, :],
                                     start=(j == 0), stop=(j == J - 1))
                if DBG and c == 0:
                    vmd = wk.tile([128, HDL], bf, name=f"vmd{c}_{tt}",
                                  tag="gv", bufs=2)
                    nc.scalar.copy(vmd[:], vm_ps[:])
                    nc.gpsimd.dma_start(
                        io["out_x1"][0:128, tt * 512:(tt + 1) * 512], vmd[:])
                    ved = wk.tile([128, HDL], bf, name=f"ved{c}_{tt}",
                                  tag="gv", bufs=2)
                    nc.scalar.copy(ved[:], ve_ps[:])
                    nc.gpsimd.dma_start(
                        io["out_x1"][128:256, tt * 512:(tt + 1) * 512], ved[:])
                    nc.gpsimd.dma_start(
                        io["out_x1"][384:512,
                                     tt * 16:tt * 16 + 1],
                        scols[c * 4 + tt][:])
                    nc.gpsimd.dma_start(
                        io["out_x1"][384:512,
                                     tt * 16 + 2:tt * 16 + 6],
                        g3s[tt][:])
                gv = wk.tile([128, HDL], bf, name=f"gv{c}_{tt}", tag="gv",
                             bufs=1)
                for h in range(HL):
                    nc.vector.tensor_scalar(
                        gv[:, h * D:(h + 1) * D], ve_ps[:, h * D:(h + 1) * D],
                        g3s[tt][:, h:h + 1], None, op0=OP.mult)
                if DBG and c == 0:
                    nc.gpsimd.dma_start(
                        io["out_x1"][256:384, tt * 512:(tt + 1) * 512], gv[:])
                nc.vector.scalar_tensor_tensor(
                    vtile[:, c * 4 + tt, :], vm_ps[:], scols[c * 4 + tt][:],
                    gv[:], op0=OP.mult, op1=OP.add)

            # ---- q projection + q/k rope + norm ----
            def rope_norm(src_ps, h, kind, dst):
                cs = cos2[:]
                sn = sin2[:]
                sb = wk.tile([128, CH], bf, name=f"{kind}sb{c}_{h}", tag="qk",
                             bufs=5)
                nc.scalar.copy(sb[:], src_ps[:])
                sq = wk.tile([128, CH], bf, name=f"{kind}sq{c}_{h}", tag="qk",
                             bufs=5)
                nc.scalar.activation(sq[:], src_ps[:], AF.Square)
                ss_ps = psp.tile([1, CH], f32, name=f"{kind}ss{c}_{h}",
                                 tag="ps", bufs=8)
                nc.tensor.matmul(ss_ps[:], onec[:], sq[:], start=True,
                                 stop=True)
                lnr = colp.tile([1, CH], bf, name=f"{kind}ln{c}_{h}",
                                tag="r512", bufs=1)
                nc.scalar.activation(lnr[:], ss_ps[:], AF.Ln,
                                     scale=qk_ln_scale, bias=eps_qk[:])
                rs2 = colp.tile([1, CH], bf, name=f"{kind}rs{c}_{h}",
                                tag="r512b", bufs=2)
                nc.scalar.activation(rs2[:], lnr[:], AF.Exp, scale=-0.5)
                rb_ps = psp.tile([128, CH], f32, name=f"{kind}rb{c}_{h}",
                                 tag="ps", bufs=8)
                nc.tensor.matmul(rb_ps[:], oner[:], rs2[:], start=True,
                                 stop=True)
                ta = wk.tile([128, CH], bf, name=f"{kind}ta{c}_{h}", tag="qk",
                             bufs=5)
                tb = wk.tile([128, CH], bf, name=f"{kind}tb{c}_{h}", tag="qk",
                             bufs=5)
                ro = wk.tile([128, CH], bf, name=f"{kind}ro{c}_{h}", tag="qk",
                             bufs=5)
                swp_ps = psp.tile([128, CH], f32, name=f"{kind}sw{c}_{h}",
                                  tag="ps", bufs=8)
                nc.tensor.matmul(swp_ps[:], p64[:], sb[:], start=True,
                                 stop=True)
                nc.vector.tensor_tensor(ta[:], sb[:], cs, op=OP.mult)
                nc.vector.tensor_tensor(tb[:], swp_ps[:], sn, op=OP.mult)
                nc.vector.tensor_tensor(ro[:], ta[:], tb[:], op=OP.add)
                nc.vector.tensor_tensor(dst, ro[:], rb_ps[:], op=OP.mult)

            qfs = []
            for h in range(HL):
                wtq = wk.tile([128, EC, 128], bf, name=f"wq{c}_{h}",
                              tag="we3", bufs=5)
                nc.sync.dma_start(
                    wtq[:], io["wqT"].rearrange("(a p) n -> p a n", p=128)
                    [:, :, h * D:(h + 1) * D])
                q_ps = psp.tile([128, CH], f32, name=f"qps{c}_{h}", tag="ps",
                                bufs=8)
                for e in range(EC):
                    nc.tensor.matmul(q_ps[:], wtq[:, e, :], xt[:, e, :],
                                     start=(e == 0), stop=(e == EC - 1))
                qf = wk.tile([128, CH], bf, name=f"qf{c}_{h}", tag="qf",
                             bufs=5)
                rope_norm(q_ps, h, "q", qf[:])
                qfs.append(qf)

                k_ps = psp.tile([128, CH], f32, name=f"kps{c}_{h}", tag="ps",
                                bufs=8)
                for j in range(J):
                    nc.tensor.matmul(k_ps[:], kI[:, h * J + j, :], kj_sb[j][:],
                                     start=(j == 0), stop=(j == J - 1))
                rope_norm(k_ps, h, "k", kT[:, h, csl])

            if c + 1 < nch:
                load_xt(c + 1)
            wot = big.tile([128, HL, E], bf, name=f"wot{c}", tag="wot", bufs=2)
            nc.sync.dma_start(
                wot[:], io["woT"].rearrange("(a p) n -> p a n", p=128))

            # ---- attention ----
            yTfs = []
            nsb = 4 * (c + 1)
            for h in range(HL):
                sums_ps = psp.tile([1, CH], f32, name=f"sums{c}_{h}", tag="ps",
                                   bufs=8)
                yT_ps = psp.tile([128, CH], f32, name=f"yT{c}_{h}", tag="ps",
                                 bufs=8)
                for sb_i in range(nsb):
                    sc_ps = psp.tile([128, CH], f32, name=f"sc{c}_{h}_{sb_i}",
                                     tag="ps", bufs=8)
                    nc.tensor.matmul(sc_ps[:],
                                     kT[:, h, sb_i * 128:(sb_i + 1) * 128],
                                     qfs[h][:], start=True, stop=True)
                    p0 = wk.tile([128, CH], bf, name=f"p0{c}_{h}_{sb_i}",
                                 tag="p", bufs=4)
                    nc.scalar.activation(p0[:], sc_ps[:], AF.Exp,
                                         scale=inv_sqrt_d)
                    if sb_i >= 4 * c:
                        pm = wk.tile([128, CH], bf, name=f"pm{c}_{h}_{sb_i}",
                                     tag="p", bufs=4)
                        nc.vector.tensor_tensor(pm[:], p0[:],
                                                masks[:, sb_i - 4 * c, :],
                                                op=OP.mult)
                    else:
                        pm = p0
                    nc.tensor.matmul(sums_ps[:], onec[:], pm[:],
                                     start=(sb_i == 0), stop=(sb_i == nsb - 1))
                    nc.tensor.matmul(yT_ps[:],
                                     vtile[:, sb_i, h * D:(h + 1) * D],
                                     pm[:], start=(sb_i == 0),
                                     stop=(sb_i == nsb - 1))
                isr = colp.tile([1, CH], bf, name=f"isr{c}_{h}", tag="r512b",
                                bufs=2)
                with nc.allow_low_precision(reason="softmax 1/sum in bf16"):
                    nc.vector.reciprocal(isr[:], sums_ps[:])
                ib_ps = psp.tile([128, CH], f32, name=f"ib{c}_{h}", tag="ps",
                                 bufs=8)
                nc.tensor.matmul(ib_ps[:], oner[:], isr[:], start=True,
                                 stop=True)
                ib = wk.tile([128, CH], bf, name=f"ibs{c}_{h}", tag="p",
                             bufs=4)
                nc.scalar.copy(ib[:], ib_ps[:])
                yTf = wk.tile([128, CH], bf, name=f"yTf{c}_{h}", tag="y",
                              bufs=4)
                nc.vector.tensor_tensor(yTf[:], yT_ps[:], ib[:], op=OP.mult)
                yTfs.append(yTf)

            # ---- Wo partial + 0.25*x, straight to AR bounce ----
            for tt in range(4):
                tsl = slice(tt * 128, (tt + 1) * 128)
                rows = slice(c * CH + tt * 128, c * CH + (tt + 1) * 128)
                xqw = wk.tile([128, E], bf, name=f"xqw{c}_{tt}", tag="xq",
                              bufs=2)
                nc.sync.dma_start(xqw[:], io["xq"][rows, :])
                for ot in range(4):
                    osl = slice(ot * 512, (ot + 1) * 512)
                    wo_ps = psp.tile([128, 512], f32,
                                     name=f"wops{c}_{tt}_{ot}", tag="ps",
                                     bufs=8)
                    for h in range(HL):
                        nc.tensor.matmul(wo_ps[:], yTfs[h][:, tsl],
                                         wot[:, h, osl], start=(h == 0),
                                         stop=(h == HL - 1))
                    aro = wk.tile([128, 512], bf, name=f"aro{c}_{tt}_{ot}",
                                  tag="p", bufs=4)
                    nc.vector.tensor_tensor(aro[:], wo_ps[:],
                                            xqw[:, osl], op=OP.add)
                    nc.sync.dma_start(cin[rows, osl], aro[:])

            if DBG and c == 0:
                # dump rows: 0-127 kT(4x512), 128-255 qf(4x512),
                # 256-383 v(4x512), 384-511 y(4x512)   [T=512 debug only]
                for h in range(HL):
                    csl2 = slice(h * CH, (h + 1) * CH)
                    nc.gpsimd.dma_start(io["out"][0:128, csl2],
                                        kT[:, h, 0:CH])
                    nc.gpsimd.dma_start(io["out"][128:256, csl2], qfs[h][:])
                    nc.gpsimd.dma_start(io["out"][256:384, csl2],
                                        vtile[:, h, :])
                    nc.gpsimd.dma_start(io["out"][384:512, csl2], yTfs[h][:])

            # ---- AllReduce this chunk within the batch group ----
            if NOAR:
                nc.sync.dma_start(cout[csl, :], cin[csl, :])
            else:
                nc.gpsimd.collective_compute(
                    "AllReduce", mybir.AluOpType.add, replica_groups=groups,
                    ins=[cin[csl, :].opt()], outs=[cout[csl, :].opt()])

            # x1 = cout chunk: s2^2 per t-tile; also forward x1 to host
            if not DBG:
                nc.sync.dma_start(io["out_x1"][csl, :], cout[csl, :])
            for tt in range(4):
                rows = slice(c * CH + tt * 128, c * CH + (tt + 1) * 128)
                x1_tt = wk.tile([128, E], bf, name=f"x1{c}_{tt}", tag="xq",
                                bufs=2)
                nc.sync.dma_start(x1_tt[:], cout[rows, :])
                msq1 = row_stats_sq(x1_tt, f"s2_{c}_{tt}")
                ln1 = colp.tile([128, 1], f32, name=f"ln1{c}_{tt}", tag="c1",
                                bufs=8)
                nc.scalar.activation(ln1[:], msq1[:], AF.Ln, scale=1.0,
                                     bias=eps_e[:])
                s2sq = colp.tile([128, 1], f32, name=f"s2sq{c}_{tt}",
                                 tag="s2col", bufs=4 * nch)
                nc.scalar.activation(s2sq[:], ln1[:], AF.Exp, scale=-1.0)
                s2cols.append(s2sq)

        # ======================= MLP phase (512-token quarters) ==========
        x1t_tiles = {}

        def load_x1t(qi):
            t = big.tile([128, EC, 512], bf, name=f"x1t{qi}", tag="xt",
                         bufs=2)
            nc.sync.dma_start_transpose(t[:], cout[qi * CH:(qi + 1) * CH, :])
            x1t_tiles[qi] = t

        if not DBG:
            load_x1t(0)
        for hf in (range(0) if DBG else range(nch)):
            t0 = hf * CH
            x1t = x1t_tiles.pop(hf)
            if hf + 1 < nch:
                load_x1t(hf + 1)

            u2s = []
            for f in range(FCT):
                wfc_f = wk.tile([128, EC, 128], bf, name=f"wfc{hf}_{f}",
                                tag="we3", bufs=5)
                nc.sync.dma_start(
                    wfc_f[:],
                    io["wfcT"].rearrange("(a p) n -> p a n", p=128)
                    [:, :, f * 128:(f + 1) * 128])
                u_ps = psp.tile([128, 512], f32, name=f"ups{hf}_{f}", tag="ps",
                                bufs=8)
                for e in range(EC):
                    nc.tensor.matmul(u_ps[:], wfc_f[:, e, :], x1t[:, e, :],
                                     start=(e == 0), stop=(e == EC - 1))
                ur = wk.tile([128, 512], bf, name=f"ur{hf}_{f}", tag="p",
                             bufs=4)
                nc.scalar.activation(ur[:], u_ps[:], AF.Relu)
                u2 = wk.tile([128, 512], bf, name=f"u2{hf}_{f}", tag="u2",
                             bufs=FCT + 1)
                nc.vector.tensor_tensor(u2[:], ur[:], ur[:], op=OP.mult)
                u2s.append(u2)

            for ot in range(4):
                osl = slice(ot * 512, (ot + 1) * 512)
                wm_ot = big.tile([128, FCT, 512], bf, name=f"wm{hf}_{ot}",
                                 tag="wot", bufs=2)
                nc.sync.dma_start(
                    wm_ot[:],
                    io["wmlpT"].rearrange("(a p) n -> p a n", p=128)[:, :, osl])
                for tl in range(4):
                    tsl = slice(tl * 128, (tl + 1) * 128)
                    mp = psp.tile([128, 512], f32, name=f"mp{hf}_{ot}_{tl}",
                                  tag="ps", bufs=8)
                    for f in range(FCT):
                        nc.tensor.matmul(mp[:], u2s[f][:, tsl], wm_ot[:, f, :],
                                         start=(f == 0), stop=(f == FCT - 1))
                    o_sb = wk.tile([128, 512], bf, name=f"o{hf}_{ot}_{tl}",
                                   tag="of", bufs=3)
                    gtt = (t0 + tl * 128) // 128
                    nc.vector.tensor_scalar(o_sb[:], mp[:], s2cols[gtt][:],
                                            None, op0=OP.mult)
                    rows = slice(t0 + tl * 128, t0 + (tl + 1) * 128)
                    nc.sync.dma_start(io["out"][rows, osl], o_sb[:])


def _pin_act_tables():
    """Force every activation onto natural_log_exp_and_others (it contains
    Exp/Ln/Square/Relu/Copy/Identity) so the table is loaded once instead of
    thrashing between per-function sets. Indices are preserved; the kept
    set's real contents are unchanged, so runtime behavior is sound."""
    import concourse.bacc as bacc_mod
    import concourse.mybir as mybir
    if getattr(bacc_mod, "_act_tables_pinned", False):
        return
    AF = mybir.ActivationFunctionType
    mine = {AF.Exp, AF.Ln, AF.Square, AF.Relu, AF.Copy, AF.Identity}
    orig = bacc_mod.get_activation_tables

    def patched(arch):
        t = orig(arch)
        out = {}
        for name, funcs in t.items():
            if name == "natural_log_exp_and_others":
                out[name] = set(funcs)
            else:
                out[name] = set(funcs) - mine
        return out

    bacc_mod.get_activation_tables = patched
    bacc_mod._act_tables_pinned = True


def build_nc(T=T_FULL, num_devices=NCORES):
    from concourse import bacc
    import concourse.tile as tile
    _pin_act_tables()
    nc = bacc.Bacc("TRN2", target_bir_lowering=False, debug=False,
                   enable_asserts=True, num_devices=num_devices)
    io = declare_io(nc, T)
    with tile.TileContext(nc) as tc:
        emit(tc, io, T)
    nc.compile()
    return nc


def combine_outputs(results, T=T_FULL):
    out = np.zeros((B, T, E), np.float32)
    for c in range(NCORES):
        out[c // 4] += np.asarray(results[c]["out"]).astype(np.float32)
    for b in range(B):
        out[b] += np.asarray(results[b * 4]["out_x1"]).astype(np.float32)
    return out


def kernel(**inputs):
    from concourse.bass_utils import run_bass_kernel_spmd
    in_maps = shard_inputs(**inputs)
    nc = build_nc(T_FULL)
    res = run_bass_kernel_spmd(nc, in_maps, core_ids=list(range(NCORES)))
    return combine_outputs(res.results, T_FULL)

